# revision 27
# baseline (speedup 1.0000x reference)
"""Trainium2 Bass kernel for nn_Metamorph_parameterReinforcer.

Math background (exact identities, verified against the reference):
  The reference's einsum("bfp,mn->bfm", fx, wfft) sums over BOTH p and n,
  so each "STFT block" collapses:
    sum_p fft(x, norm=forward)[..., p] == x[..., 0]
    block(x)[b, f, k] = Re tanh(x[b, f, 0] * W[k]),
       W[k] = sum_m (sum_n wfft[m, n]) * exp(2j*pi*k*m/64)
  Chaining three blocks, only element 0 of the last axis propagates:
    a  = params[:, :, 0]
    s1 = Retanh(a  * W0[0]);  s2 = Retanh(s1 * W1[0])
    x3[b, f, l] = Retanh(s2[b, f] * W2[l])         # (512, 1000, 64)
    h  = tanh(x3.reshape(512, 64000) @ lin1_w.T + lin1_b)
    out = sigmoid(h @ lin2_w.T + lin2_b)
  Because |W0[0]|, |W1[0]| ~ 32000, tanh saturates and s2 is +-1 in f32 for
  all but (rare) tiny-|a| entries; where s2 = +-1, x3[b, f, :] =
  s2[b, f] * X1[:] with X1 = Retanh(W2) -- exactly rank-1.  Rare
  non-saturated entries get an exact correction (dht) added before the lin1
  tanh.

Device kernel (8 cores, lin1_w sharded over its output dim j, 125 rows/core).
v2 design (fp8 + DoubleRow + constant stationary operand):
  * X1 is folded into lin1_w on the host: w1'[o, f, l] = lin1_w[o, 64f+l] *
    X1[l].  Stage 1 then reduces w1' over l with a CONSTANT ones
    block-diagonal stationary operand, so every matmul shares one lhsT and
    columns from different f-groups ride in the same matmul.
  * w1' is quantized to fp8e4 (TRN E4M3, max 240) with ERROR-FEEDBACK along
    l: the rounding error of element l is carried into element l+1, so the
    per-(o,f) SUM over l is accurate to ~1 ulp instead of sqrt(64) ulps.
    Simulated end-to-end rel err 2.4e-3 (vs 1.7e-2 naive fp8, 1.5e-3 bf16).
  * DoubleRow perf mode: virtual K=256 (2 fp8 per PE cell), halving both
    DMA bytes (8.4 MB/core) and PE streaming time vs bf16.  DoubleRow
    outputs must sit at psum partition 0 (walrus ISA check), so:
  * f padded 1000->1024, split into 8 banks of 128.  Bank B accumulates
    psum [32, 500] over 8 passes: psum[m, 125*c4 + j] = A[f, j] with
    f = 128B + 32*c4 + m.  One chunk = one bank's full fp8 stream (1 MB).
  * Per bank: DVE/ScalarE copy [32, 500] psum->SBUF (bf16), then ONE
    gpsimd SBUF->SBUF DMA reshapes [32, 500] -> [128, 125] (partition p =
    4m + c4; the host permutes s2t rows to match), giving the [128 f, 125 j]
    lhsT tile for stage 2.  All per-bank work overlaps later banks' stream.
  * stage 2: h[j, b] = tanh(2^-11 * sum_f A[f, j] s2[f, b] + bias) -- 8
    accumulating matmuls (K = 128 f's each) + ScalarE tanh (the activation
    scale folds the fp8 2^11 quantization scale away).
  * stage 3: partial[k, b] = sum_j l2t[j, k] h[j, b]  (one matmul)
Host combines the 8 partials: out = sigmoid(sum_c partial_c + lin2_b).
"""

import numpy as np

B, MODES, L = 512, 1000, 64
NCORES = 8
JSH = MODES // NCORES          # 125 lin1 output rows per core
FPAD = 1024                    # padded mode count (8 stage-2 K-tiles of 128)
NCHUNK = 8                     # w1x DMA chunks = psum banks (1 MB each)
CHCOL = 4000                   # columns per chunk: (P, c4, j) = 8*4*125
SCALE = 2.0 ** 11              # fp8 quantization scale for w1'
SAT = 50.0                     # |2*s*Re(W)| beyond this: Retanh == sign


def _retanh(s, w):
    """Re tanh(s * w) for real array s and complex (array or scalar) w."""
    s = np.asarray(s, np.float64)
    x = 2.0 * np.multiply.outer(s, np.real(w))
    y = 2.0 * np.multiply.outer(s, np.imag(w))
    xc = np.clip(x, -SAT, SAT)
    with np.errstate(over="ignore", invalid="ignore"):
        r = np.sinh(xc) / (np.cosh(xc) + np.cos(y))
    return np.where(np.abs(x) >= SAT, np.sign(x), r)


def _wvec(wre, wim):
    """W[k] = sum_m (sum_n w[m, n]) * exp(2j pi k m / L)."""
    wsum = wre.astype(np.float64).sum(axis=1) + 1j * wim.astype(np.float64).sum(axis=1)
    tw = np.exp(2j * np.pi * np.outer(np.arange(L), np.arange(L)) / L)
    return tw @ wsum


_CACHE = {}


def _build_program(use_dh):
    """Build (and cache) the Bass program. Same program for all 8 cores."""
    key = ("prog", use_dh, "fp8dr_v8")
    if key in _CACHE:
        return _CACHE[key]

    import concourse.bacc as bacc
    import concourse.mybir as mybir
    import concourse.tile as tile

    f32 = mybir.dt.float32
    f32r = mybir.dt.float32r
    bf16 = mybir.dt.bfloat16
    fp8 = mybir.dt.float8e4
    DR = mybir.MatmulPerfMode.DoubleRow
    nc = bacc.Bacc("TRN2", target_bir_lowering=False, debug=False)

    # [p, bank, (P, i), c4*j]: each (p, bank) is one contiguous 8 KB run in
    # DRAM, and each (p, bank, pass-half) is a contiguous 4 KB run, so both
    # full-bank and half-bank DMAs keep fat descriptors (HWDGE descriptor
    # generation is the ring-throughput limit for runs under ~4 KB).
    w1x_d = nc.dram_tensor("w1x", [NCHUNK, 128, 8000], fp8, kind="ExternalInput")
    s2t_d = nc.dram_tensor("s2t", [128, 6 * B], fp8, kind="ExternalInput")
    # banks 6-7's s2 rows live on partitions 0-31 for the direct K=32
    # stage-2 (saves the reshape DMA latency on the critical tail)
    s2l_d = nc.dram_tensor("s2l", [32, 8 * B], fp8, kind="ExternalInput")
    ones_d = nc.dram_tensor("ones8", [128, 2, 32], fp8, kind="ExternalInput")
    bias_d = nc.dram_tensor("bias", [JSH, 1], f32, kind="ExternalInput")
    l2t_d = nc.dram_tensor("l2t", [JSH, L], f32r, kind="ExternalInput")
    if use_dh:
        dht_d = nc.dram_tensor("dht", [JSH, B], f32, kind="ExternalInput")
    outp_d = nc.dram_tensor("outp", [L, B], f32, kind="ExternalOutput")

    with tile.TileContext(nc) as tc:
        with (
            tc.tile_pool(name="const", bufs=1) as const,
            tc.tile_pool(name="w1pool", bufs=3) as w1pool,
            tc.tile_pool(name="s4pool", bufs=2) as s4pool,
            tc.tile_pool(name="acc", bufs=1) as acc,
            tc.tile_pool(name="psA", bufs=3, space="PSUM") as psA,
            tc.tile_pool(name="psH", bufs=1, space="PSUM") as psH,
            tc.tile_pool(name="psO", bufs=1, space="PSUM") as psO,
        ):
            # ones8 first on the sync ring (needed by the first LDWEIGHTS);
            # all other consts are issued AFTER the w1x chunks (they are only
            # needed by stage 2/3).  s2t/s2l ride the SWDGE queue.
            ones8 = const.tile([128, 2, 32], fp8)
            nc.sync.dma_start(ones8[:], ones_d.ap())
            s2t = const.tile([128, 6 * B], fp8)
            nc.gpsimd.dma_start(s2t[:], s2t_d.ap())
            s2l = const.tile([32, 8 * B], fp8)
            nc.gpsimd.dma_start(s2l[:], s2l_d.ap())

            at_sb = acc.tile([128, 6 * JSH], bf16)
            ph = psH.tile([JSH, B], f32)

            dma_engines = [nc.sync, nc.scalar]

            for bk in range(NCHUNK):
                # two pass-range halves striped across BOTH HWDGE rings:
                # banks arrive strictly in order at the full aggregate rate
                w1c = w1pool.tile([128, 8000], fp8, tag="w1c")
                for hf in range(2):
                    dma_engines[hf].dma_start(
                        w1c[:, 4000 * hf : 4000 * hf + 4000],
                        w1x_d.ap()[bk, :, 4000 * hf : 4000 * hf + 4000],
                    )
                pa = psA.tile([32, 512], f32, name="pa", tag="pa")
                for p8 in range(8):
                    nc.tensor.matmul(
                        pa[:, 0:500],
                        ones8[:],
                        w1c[:, 1000 * p8 : 1000 * p8 + 1000].rearrange(
                            "p (i c) -> p i c", i=2
                        ),
                        start=(p8 == 0),
                        stop=(p8 == 7),
                        perf_mode=DR,
                    )
                # psum [32, 500] -> SBUF bf16
                s4 = s4pool.tile([32, 500], bf16, tag="s4")
                if bk % 2 == 0:
                    nc.vector.tensor_copy(s4[:], pa[:, 0:500])
                else:
                    nc.scalar.activation(
                        s4[:], pa[:, 0:500], mybir.ActivationFunctionType.Copy
                    )
                if bk < 6:
                    # reshape-DMA to the [128, 125] stage-2 lhsT tile
                    # (partition p = 4m + c4), then one K=128 matmul
                    nc.gpsimd.dma_start(at_sb[:, JSH * bk : JSH * (bk + 1)], s4[:])
                    nc.tensor.matmul(
                        ph[:, :],
                        at_sb[:, JSH * bk : JSH * (bk + 1)],
                        s2t[:, B * bk : B * (bk + 1)],
                        start=(bk == 0),
                        stop=False,
                    )
                else:
                    # last two banks: direct K=32 matmuls from s4 (no
                    # reshape DMA on the critical tail)
                    for c4 in range(4):
                        g = 4 * (bk - 6) + c4
                        nc.tensor.matmul(
                            ph[:, :],
                            s4[:, JSH * c4 : JSH * (c4 + 1)],
                            s2l[:, B * g : B * (g + 1)],
                            start=False,
                            stop=(bk == 7 and c4 == 3),
                        )

            # late consts (needed only for the activation / stage 3)
            bias = const.tile([JSH, 1], f32)
            nc.sync.dma_start(bias[:], bias_d.ap())
            l2t = const.tile([JSH, L], f32r)
            nc.sync.dma_start(l2t[:], l2t_d.ap())
            if use_dh:
                dht = const.tile([JSH, B], f32)
                nc.sync.dma_start(dht[:], dht_d.ap())
                nc.vector.tensor_add(ph[:, :], ph[:, :], dht[:, :])

            # ---- tanh + stage 3 + output, pipelined in two column halves
            # (separate psum banks so the zero regions stay independent) ----
            h_sb = acc.tile([JSH, B], f32r)
            o_sb = acc.tile([L, B], f32)
            H = B // 2
            for hf in range(2):
                cs = slice(H * hf, H * (hf + 1))
                nc.scalar.activation(
                    h_sb[:, cs],
                    ph[:, cs],
                    mybir.ActivationFunctionType.Tanh,
                    bias=bias[:, 0:1],
                    scale=float(1.0 / SCALE),
                )
                po = psO.tile([L, H], f32, name=f"po{hf}", tag=f"po{hf}")
                nc.tensor.matmul(
                    po[:, :], l2t[:, :], h_sb[:, cs], start=True, stop=True
                )
                nc.vector.tensor_copy(o_sb[:, cs], po[:, :])
                nc.sync.dma_start(outp_d.ap()[:, cs], o_sb[:, cs])

    nc.compile()
    _CACHE[key] = nc
    return nc


def _quantize_feedback(w1p):
    """fp8e4 (TRN E4M3) quantization with error feedback along the last axis.

    Carrying each element's rounding error into the next keeps the sum over
    the last axis accurate to ~1 ulp of a single element.
    """
    import ml_dtypes

    fp8 = ml_dtypes.float8_e4m3
    q = np.empty(w1p.shape, dtype=fp8)
    err = np.zeros(w1p.shape[:-1], np.float32)
    for l in range(w1p.shape[-1]):
        v = w1p[..., l] + err
        ql = np.clip(v, -240, 240).astype(fp8)
        q[..., l] = ql
        err = v - ql.astype(np.float32)
    return q


def profile_last(trace_cores=None):
    """Re-run the last-built program with NTFF tracing (dev/test helper)."""
    if "last_run" not in _CACHE:
        return None
    from concourse.bass_utils import run_bass_kernel_spmd

    nc, in_maps = _CACHE["last_run"]
    return run_bass_kernel_spmd(
        nc,
        in_maps,
        list(range(NCORES)),
        trace=True,
        trace_cores=trace_cores,
    )


def _host_prep(
    params,
    wfft0_re,
    wfft0_im,
    wfft1_re,
    wfft1_im,
    wfft2_re,
    wfft2_im,
    lin1_w,
    lin1_b,
    lin2_w,
    lin2_b,
):
    """All host-side prep: collapse, quantize, per-core shards."""
    import ml_dtypes

    fp8 = ml_dtypes.float8_e4m3
    bf16 = ml_dtypes.bfloat16

    a = params[:, :, 0].astype(np.float64)
    w0 = _wvec(wfft0_re, wfft0_im)[0]
    w1v = _wvec(wfft1_re, wfft1_im)[0]
    w2 = _wvec(wfft2_re, wfft2_im)
    s1 = _retanh(a, w0)
    s2 = _retanh(s1, w1v)                          # (B, MODES) f64
    x1 = _retanh(np.float64(1.0), w2)              # (64,) f64

    # s2 as the device will see it (fp8 in the stage-2 rhs; +-1 is exact)
    s2q = np.clip(s2, -240, 240).astype(fp8).astype(np.float32)

    # exact correction for entries where tanh did not saturate to +-1
    bad_b, bad_f = np.nonzero(np.abs(s2q) != np.float32(1.0))
    use_dh = bad_b.size > 0
    dh = None
    if use_dh:
        dh = np.zeros((B, MODES), np.float64)
        x1_64 = x1.astype(np.float64)
        for bb, ff in zip(bad_b.tolist(), bad_f.tolist()):
            sdev = np.float64(s2q[bb, ff])
            delta = _retanh(s2[bb, ff], w2) - sdev * x1_64   # (64,)
            dh[bb, :] += lin1_w[:, 64 * ff : 64 * (ff + 1)].astype(np.float64) @ delta
        dh = (dh * SCALE).astype(np.float32)       # pre-scaled like the psum

    # ---- fold X1 into lin1_w, quantize fp8 with error feedback ----
    w1p = (
        lin1_w.reshape(MODES, MODES, L).astype(np.float32)
        * x1[None, None, :].astype(np.float32)
    ) * np.float32(SCALE)                          # (o, f, l)
    w1q = _quantize_feedback(w1p)                  # (o, f, l) fp8
    del w1p

    # s2t layout: [128 p, 8 B, 512 b]; tile B row p holds f = 128B + 32*(p%4) + p//4
    # (that is where the [32,500]->[128,125] reshape DMA puts A[f, :])
    p_idx = np.arange(128)
    f_of_p = 32 * (p_idx % 4) + p_idx // 4          # within-tile f offset
    s2f = np.zeros((FPAD, B), np.float32)
    s2f[:MODES] = s2q.T
    s2t = np.zeros((128, 6, B), fp8)
    for t in range(6):
        s2t[:, t, :] = s2f[128 * t + f_of_p].astype(fp8)
    s2t = np.ascontiguousarray(s2t.reshape(128, 6 * B))
    # banks 6-7 (direct K=32 stage-2): s2l[m, g, b] = s2f[768 + 32 g + m, b]
    s2l = np.ascontiguousarray(
        s2f[768:1024].reshape(8, 32, B).transpose(1, 0, 2).astype(fp8).reshape(32, 8 * B)
    )

    # ones block-diagonal stationary operand: ones8[p, i, m] = (m == 16i + p//8)
    ones8 = np.zeros((128, 2, 32), np.float32)
    for p in range(128):
        for i in range(2):
            ones8[p, i, 16 * i + p // 8] = 1.0
    ones8 = ones8.astype(fp8)

    in_maps = []
    for c in range(NCORES):
        j0, j1 = JSH * c, JSH * (c + 1)
        # w1x[p, B, (P, i), (c4, j)]:
        #   f = 128B + 32c4 + 16i + p//8,  l = 8P + p%8
        qp = np.zeros((JSH, FPAD, L), fp8)
        qp[:, :MODES, :] = w1q[j0:j1]
        w1x = np.ascontiguousarray(
            qp.reshape(JSH, 8, 4, 2, 16, 8, 8)         # j B c4 i mh P dl
            .transpose(1, 4, 6, 5, 3, 2, 0)            # B mh dl P i c4 j
            .reshape(NCHUNK, 128, 8000)
        )
        m = {
            "w1x": w1x,
            "s2t": s2t,
            "s2l": s2l,
            "ones8": ones8,
            "bias": np.ascontiguousarray(lin1_b[j0:j1].reshape(JSH, 1)),
            "l2t": np.ascontiguousarray(lin2_w[:, j0:j1].T),
        }
        if use_dh:
            m["dht"] = np.ascontiguousarray(dh[:, j0:j1].T)
        in_maps.append(m)
    return use_dh, in_maps


def kernel(
    params,
    wfft0_re,
    wfft0_im,
    wfft1_re,
    wfft1_im,
    wfft2_re,
    wfft2_im,
    lin1_w,
    lin1_b,
    lin2_w,
    lin2_b,
):
    from concourse.bass_utils import run_bass_kernel_spmd

    use_dh, in_maps = _host_prep(
        params, wfft0_re, wfft0_im, wfft1_re, wfft1_im, wfft2_re, wfft2_im,
        lin1_w, lin1_b, lin2_w, lin2_b,
    )
    nc = _build_program(use_dh)
    _CACHE["last_run"] = (nc, in_maps)
    res = run_bass_kernel_spmd(nc, in_maps, list(range(NCORES)))

    acc = np.zeros((L, B), np.float64)
    for c in range(NCORES):
        acc += res.results[c]["outp"].astype(np.float64)
    out = 1.0 / (1.0 + np.exp(-(acc.T + lin2_b.astype(np.float64))))
    return out.astype(np.float32)


# revision 28
# speedup vs baseline: 1.0741x; 1.0741x over previous
"""Trainium2 Bass kernel for nn_Metamorph_parameterReinforcer.

Math background (exact identities, verified against the reference):
  The reference's einsum("bfp,mn->bfm", fx, wfft) sums over BOTH p and n,
  so each "STFT block" collapses:
    sum_p fft(x, norm=forward)[..., p] == x[..., 0]
    block(x)[b, f, k] = Re tanh(x[b, f, 0] * W[k]),
       W[k] = sum_m (sum_n wfft[m, n]) * exp(2j*pi*k*m/64)
  Chaining three blocks, only element 0 of the last axis propagates:
    a  = params[:, :, 0]
    s1 = Retanh(a  * W0[0]);  s2 = Retanh(s1 * W1[0])
    x3[b, f, l] = Retanh(s2[b, f] * W2[l])         # (512, 1000, 64)
    h  = tanh(x3.reshape(512, 64000) @ lin1_w.T + lin1_b)
    out = sigmoid(h @ lin2_w.T + lin2_b)
  Because |W0[0]|, |W1[0]| ~ 32000, tanh saturates and s2 is +-1 in f32 for
  all but (rare) tiny-|a| entries; where s2 = +-1, x3[b, f, :] =
  s2[b, f] * X1[:] with X1 = Retanh(W2) -- exactly rank-1.  Rare
  non-saturated entries get an exact correction (dht) added before the lin1
  tanh.

Device kernel (8 cores, lin1_w sharded over its output dim j, 125 rows/core).
v2 design (fp8 + DoubleRow + constant stationary operand):
  * X1 is folded into lin1_w on the host: w1'[o, f, l] = lin1_w[o, 64f+l] *
    X1[l].  Stage 1 then reduces w1' over l with a CONSTANT ones
    block-diagonal stationary operand, so every matmul shares one lhsT and
    columns from different f-groups ride in the same matmul.
  * w1' is quantized to fp8e4 (TRN E4M3, max 240) with ERROR-FEEDBACK along
    l: the rounding error of element l is carried into element l+1, so the
    per-(o,f) SUM over l is accurate to ~1 ulp instead of sqrt(64) ulps.
    Simulated end-to-end rel err 2.4e-3 (vs 1.7e-2 naive fp8, 1.5e-3 bf16).
  * DoubleRow perf mode: virtual K=256 (2 fp8 per PE cell), halving both
    DMA bytes (8.4 MB/core) and PE streaming time vs bf16.  DoubleRow
    outputs must sit at psum partition 0 (walrus ISA check), so:
  * f padded 1000->1024, split into 8 banks of 128.  Bank B accumulates
    psum [32, 500] over 8 passes: psum[m, 125*c4 + j] = A[f, j] with
    f = 128B + 32*c4 + m.  One chunk = one bank's full fp8 stream (1 MB).
  * Per bank: DVE/ScalarE copy [32, 500] psum->SBUF (bf16), then ONE
    gpsimd SBUF->SBUF DMA reshapes [32, 500] -> [128, 125] (partition p =
    4m + c4; the host permutes s2t rows to match), giving the [128 f, 125 j]
    lhsT tile for stage 2.  All per-bank work overlaps later banks' stream.
  * stage 2: h[j, b] = tanh(2^-11 * sum_f A[f, j] s2[f, b] + bias) -- 8
    accumulating matmuls (K = 128 f's each) + ScalarE tanh (the activation
    scale folds the fp8 2^11 quantization scale away).
  * stage 3: partial[k, b] = sum_j l2t[j, k] h[j, b]  (one matmul)
Host combines the 8 partials: out = sigmoid(sum_c partial_c + lin2_b).
"""

import numpy as np

B, MODES, L = 512, 1000, 64
NCORES = 8
JSH = MODES // NCORES          # 125 lin1 output rows per core
FPAD = 1024                    # padded mode count (8 stage-2 K-tiles of 128)
NCHUNK = 8                     # w1x DMA chunks = psum banks (1 MB each)
CHCOL = 4000                   # columns per chunk: (P, c4, j) = 8*4*125
SCALE = 2.0 ** 11              # fp8 quantization scale for w1'
SAT = 50.0                     # |2*s*Re(W)| beyond this: Retanh == sign


def _retanh(s, w):
    """Re tanh(s * w) for real array s and complex (array or scalar) w."""
    s = np.asarray(s, np.float64)
    x = 2.0 * np.multiply.outer(s, np.real(w))
    y = 2.0 * np.multiply.outer(s, np.imag(w))
    xc = np.clip(x, -SAT, SAT)
    with np.errstate(over="ignore", invalid="ignore"):
        r = np.sinh(xc) / (np.cosh(xc) + np.cos(y))
    return np.where(np.abs(x) >= SAT, np.sign(x), r)


def _wvec(wre, wim):
    """W[k] = sum_m (sum_n w[m, n]) * exp(2j pi k m / L)."""
    wsum = wre.astype(np.float64).sum(axis=1) + 1j * wim.astype(np.float64).sum(axis=1)
    tw = np.exp(2j * np.pi * np.outer(np.arange(L), np.arange(L)) / L)
    return tw @ wsum


_CACHE = {}


def _build_program(use_dh):
    """Build (and cache) the Bass program. Same program for all 8 cores."""
    key = ("prog", use_dh, "fp8dr_v9")
    if key in _CACHE:
        return _CACHE[key]

    import concourse.bacc as bacc
    import concourse.mybir as mybir
    import concourse.tile as tile

    f32 = mybir.dt.float32
    f32r = mybir.dt.float32r
    bf16 = mybir.dt.bfloat16
    fp8 = mybir.dt.float8e4
    DR = mybir.MatmulPerfMode.DoubleRow
    nc = bacc.Bacc("TRN2", target_bir_lowering=False, debug=False)

    # [p, bank, (P, i), c4*j]: each (p, bank) is one contiguous 8 KB run in
    # DRAM, and each (p, bank, pass-half) is a contiguous 4 KB run, so both
    # full-bank and half-bank DMAs keep fat descriptors (HWDGE descriptor
    # generation is the ring-throughput limit for runs under ~4 KB).
    w1x_d = nc.dram_tensor("w1x", [NCHUNK, 128, 8000], fp8, kind="ExternalInput")
    s2t_d = nc.dram_tensor("s2t", [128, 6 * B], fp8, kind="ExternalInput")
    # banks 6-7's s2 rows live on partitions 0-31 for the direct K=32
    # stage-2 (saves the reshape DMA latency on the critical tail)
    s2l_d = nc.dram_tensor("s2l", [32, 8 * B], fp8, kind="ExternalInput")
    ones_d = nc.dram_tensor("ones8", [128, 2, 32], fp8, kind="ExternalInput")
    bias_d = nc.dram_tensor("bias", [JSH, 1], f32, kind="ExternalInput")
    l2t_d = nc.dram_tensor("l2t", [JSH, L], f32r, kind="ExternalInput")
    if use_dh:
        dht_d = nc.dram_tensor("dht", [JSH, B], f32, kind="ExternalInput")
    outp_d = nc.dram_tensor("outp", [L, B], f32, kind="ExternalOutput")

    with tile.TileContext(nc) as tc:
        with (
            tc.tile_pool(name="const", bufs=1) as const,
            tc.tile_pool(name="w1pool", bufs=5) as w1pool,
            tc.tile_pool(name="s4pool", bufs=3) as s4pool,
            tc.tile_pool(name="acc", bufs=1) as acc,
            tc.tile_pool(name="psA", bufs=3, space="PSUM") as psA,
            tc.tile_pool(name="psH", bufs=1, space="PSUM") as psH,
            tc.tile_pool(name="psO", bufs=1, space="PSUM") as psO,
        ):
            # ones8 rides the scalar ring so the sync ring's first
            # descriptor-gen is bank 0's first half; all other consts are
            # issued AFTER the w1x chunks (they are only needed by stage
            # 2/3).  s2t/s2l ride the SWDGE queue.
            ones8 = const.tile([128, 2, 32], fp8)
            nc.scalar.dma_start(ones8[:], ones_d.ap())
            s2t = const.tile([128, 6 * B], fp8)
            nc.gpsimd.dma_start(s2t[:], s2t_d.ap())
            s2l = const.tile([32, 8 * B], fp8)
            nc.gpsimd.dma_start(s2l[:], s2l_d.ap())

            at_sb = acc.tile([128, 6 * JSH], bf16)
            ph = psH.tile([JSH, B], f32)

            dma_engines = [nc.sync, nc.scalar]

            for bk in range(NCHUNK):
                # two pass-range halves striped across BOTH HWDGE rings:
                # banks arrive strictly in order at the full aggregate rate
                w1c = w1pool.tile([128, 8000], fp8, tag="w1c")
                for hf in range(2):
                    dma_engines[hf].dma_start(
                        w1c[:, 4000 * hf : 4000 * hf + 4000],
                        w1x_d.ap()[bk, :, 4000 * hf : 4000 * hf + 4000],
                    )
                pa = psA.tile([32, 512], f32, name="pa", tag="pa")
                for p8 in range(8):
                    nc.tensor.matmul(
                        pa[:, 0:500],
                        ones8[:],
                        w1c[:, 1000 * p8 : 1000 * p8 + 1000].rearrange(
                            "p (i c) -> p i c", i=2
                        ),
                        start=(p8 == 0),
                        stop=(p8 == 7),
                        perf_mode=DR,
                    )
                # psum [32, 500] -> SBUF bf16
                s4 = s4pool.tile([32, 500], bf16, tag="s4")
                if bk % 2 == 0:
                    nc.vector.tensor_copy(s4[:], pa[:, 0:500])
                else:
                    nc.scalar.activation(
                        s4[:], pa[:, 0:500], mybir.ActivationFunctionType.Copy
                    )
                if bk < 6:
                    # reshape-DMA to the [128, 125] stage-2 lhsT tile
                    # (partition p = 4m + c4), then one K=128 matmul
                    nc.gpsimd.dma_start(at_sb[:, JSH * bk : JSH * (bk + 1)], s4[:])
                    nc.tensor.matmul(
                        ph[:, :],
                        at_sb[:, JSH * bk : JSH * (bk + 1)],
                        s2t[:, B * bk : B * (bk + 1)],
                        start=(bk == 0),
                        stop=False,
                    )
                else:
                    # last two banks: direct K=32 matmuls from s4 (no
                    # reshape DMA on the critical tail)
                    for c4 in range(4):
                        g = 4 * (bk - 6) + c4
                        nc.tensor.matmul(
                            ph[:, :],
                            s4[:, JSH * c4 : JSH * (c4 + 1)],
                            s2l[:, B * g : B * (g + 1)],
                            start=False,
                            stop=(bk == 7 and c4 == 3),
                        )

            # late consts (needed only for the activation / stage 3)
            bias = const.tile([JSH, 1], f32)
            nc.sync.dma_start(bias[:], bias_d.ap())
            l2t = const.tile([JSH, L], f32r)
            nc.sync.dma_start(l2t[:], l2t_d.ap())
            if use_dh:
                dht = const.tile([JSH, B], f32)
                nc.sync.dma_start(dht[:], dht_d.ap())
                nc.vector.tensor_add(ph[:, :], ph[:, :], dht[:, :])

            # ---- tanh + stage 3 + output, pipelined in two column halves
            # (separate psum banks so the zero regions stay independent) ----
            h_sb = acc.tile([JSH, B], f32r)
            o_sb = acc.tile([L, B], f32)
            H = B // 2
            for hf in range(2):
                cs = slice(H * hf, H * (hf + 1))
                nc.scalar.activation(
                    h_sb[:, cs],
                    ph[:, cs],
                    mybir.ActivationFunctionType.Tanh,
                    bias=bias[:, 0:1],
                    scale=float(1.0 / SCALE),
                )
                po = psO.tile([L, H], f32, name=f"po{hf}", tag=f"po{hf}")
                nc.tensor.matmul(
                    po[:, :], l2t[:, :], h_sb[:, cs], start=True, stop=True
                )
                nc.vector.tensor_copy(o_sb[:, cs], po[:, :])
                nc.sync.dma_start(outp_d.ap()[:, cs], o_sb[:, cs])

    nc.compile()
    _CACHE[key] = nc
    return nc


def _quantize_feedback(w1p):
    """fp8e4 (TRN E4M3) quantization with error feedback along the last axis.

    Carrying each element's rounding error into the next keeps the sum over
    the last axis accurate to ~1 ulp of a single element.
    """
    import ml_dtypes

    fp8 = ml_dtypes.float8_e4m3
    q = np.empty(w1p.shape, dtype=fp8)
    err = np.zeros(w1p.shape[:-1], np.float32)
    for l in range(w1p.shape[-1]):
        v = w1p[..., l] + err
        ql = np.clip(v, -240, 240).astype(fp8)
        q[..., l] = ql
        err = v - ql.astype(np.float32)
    return q


def profile_last(trace_cores=None):
    """Re-run the last-built program with NTFF tracing (dev/test helper)."""
    if "last_run" not in _CACHE:
        return None
    from concourse.bass_utils import run_bass_kernel_spmd

    nc, in_maps = _CACHE["last_run"]
    return run_bass_kernel_spmd(
        nc,
        in_maps,
        list(range(NCORES)),
        trace=True,
        trace_cores=trace_cores,
    )


def _host_prep(
    params,
    wfft0_re,
    wfft0_im,
    wfft1_re,
    wfft1_im,
    wfft2_re,
    wfft2_im,
    lin1_w,
    lin1_b,
    lin2_w,
    lin2_b,
):
    """All host-side prep: collapse, quantize, per-core shards."""
    import ml_dtypes

    fp8 = ml_dtypes.float8_e4m3
    bf16 = ml_dtypes.bfloat16

    a = params[:, :, 0].astype(np.float64)
    w0 = _wvec(wfft0_re, wfft0_im)[0]
    w1v = _wvec(wfft1_re, wfft1_im)[0]
    w2 = _wvec(wfft2_re, wfft2_im)
    s1 = _retanh(a, w0)
    s2 = _retanh(s1, w1v)                          # (B, MODES) f64
    x1 = _retanh(np.float64(1.0), w2)              # (64,) f64

    # s2 as the device will see it (fp8 in the stage-2 rhs; +-1 is exact)
    s2q = np.clip(s2, -240, 240).astype(fp8).astype(np.float32)

    # exact correction for entries where tanh did not saturate to +-1
    bad_b, bad_f = np.nonzero(np.abs(s2q) != np.float32(1.0))
    use_dh = bad_b.size > 0
    dh = None
    if use_dh:
        dh = np.zeros((B, MODES), np.float64)
        x1_64 = x1.astype(np.float64)
        for bb, ff in zip(bad_b.tolist(), bad_f.tolist()):
            sdev = np.float64(s2q[bb, ff])
            delta = _retanh(s2[bb, ff], w2) - sdev * x1_64   # (64,)
            dh[bb, :] += lin1_w[:, 64 * ff : 64 * (ff + 1)].astype(np.float64) @ delta
        dh = (dh * SCALE).astype(np.float32)       # pre-scaled like the psum

    # ---- fold X1 into lin1_w, quantize fp8 with error feedback ----
    w1p = (
        lin1_w.reshape(MODES, MODES, L).astype(np.float32)
        * x1[None, None, :].astype(np.float32)
    ) * np.float32(SCALE)                          # (o, f, l)
    w1q = _quantize_feedback(w1p)                  # (o, f, l) fp8
    del w1p

    # s2t layout: [128 p, 8 B, 512 b]; tile B row p holds f = 128B + 32*(p%4) + p//4
    # (that is where the [32,500]->[128,125] reshape DMA puts A[f, :])
    p_idx = np.arange(128)
    f_of_p = 32 * (p_idx % 4) + p_idx // 4          # within-tile f offset
    s2f = np.zeros((FPAD, B), np.float32)
    s2f[:MODES] = s2q.T
    s2t = np.zeros((128, 6, B), fp8)
    for t in range(6):
        s2t[:, t, :] = s2f[128 * t + f_of_p].astype(fp8)
    s2t = np.ascontiguousarray(s2t.reshape(128, 6 * B))
    # banks 6-7 (direct K=32 stage-2): s2l[m, g, b] = s2f[768 + 32 g + m, b]
    s2l = np.ascontiguousarray(
        s2f[768:1024].reshape(8, 32, B).transpose(1, 0, 2).astype(fp8).reshape(32, 8 * B)
    )

    # ones block-diagonal stationary operand: ones8[p, i, m] = (m == 16i + p//8)
    ones8 = np.zeros((128, 2, 32), np.float32)
    for p in range(128):
        for i in range(2):
            ones8[p, i, 16 * i + p // 8] = 1.0
    ones8 = ones8.astype(fp8)

    in_maps = []
    for c in range(NCORES):
        j0, j1 = JSH * c, JSH * (c + 1)
        # w1x[p, B, (P, i), (c4, j)]:
        #   f = 128B + 32c4 + 16i + p//8,  l = 8P + p%8
        qp = np.zeros((JSH, FPAD, L), fp8)
        qp[:, :MODES, :] = w1q[j0:j1]
        w1x = np.ascontiguousarray(
            qp.reshape(JSH, 8, 4, 2, 16, 8, 8)         # j B c4 i mh P dl
            .transpose(1, 4, 6, 5, 3, 2, 0)            # B mh dl P i c4 j
            .reshape(NCHUNK, 128, 8000)
        )
        m = {
            "w1x": w1x,
            "s2t": s2t,
            "s2l": s2l,
            "ones8": ones8,
            "bias": np.ascontiguousarray(lin1_b[j0:j1].reshape(JSH, 1)),
            "l2t": np.ascontiguousarray(lin2_w[:, j0:j1].T),
        }
        if use_dh:
            m["dht"] = np.ascontiguousarray(dh[:, j0:j1].T)
        in_maps.append(m)
    return use_dh, in_maps


def kernel(
    params,
    wfft0_re,
    wfft0_im,
    wfft1_re,
    wfft1_im,
    wfft2_re,
    wfft2_im,
    lin1_w,
    lin1_b,
    lin2_w,
    lin2_b,
):
    from concourse.bass_utils import run_bass_kernel_spmd

    use_dh, in_maps = _host_prep(
        params, wfft0_re, wfft0_im, wfft1_re, wfft1_im, wfft2_re, wfft2_im,
        lin1_w, lin1_b, lin2_w, lin2_b,
    )
    nc = _build_program(use_dh)
    _CACHE["last_run"] = (nc, in_maps)
    res = run_bass_kernel_spmd(nc, in_maps, list(range(NCORES)))

    acc = np.zeros((L, B), np.float64)
    for c in range(NCORES):
        acc += res.results[c]["outp"].astype(np.float64)
    out = 1.0 / (1.0 + np.exp(-(acc.T + lin2_b.astype(np.float64))))
    return out.astype(np.float32)
